# revision 10
# baseline (speedup 1.0000x reference)
"""Trainium2 Bass kernel for BaichuanAttention (hidden=5120, 40 heads, b=2, s=2048).

Tensor-parallel over heads across 8 NeuronCores, bf16 compute:
  A) QKV projection with SBUF-resident bf16 weights, X streamed.
  B) Flash-style causal attention in S^T form (scores computed as K^T.Q so
     exp() writes P^T directly -- no P transposes), V transposed on-chip.
     Score matmuls of chunk qc are interleaved with the PV matmuls of chunk
     qc-1 in PE program order so the exp() latency never gates the PE.
  C) Softmax-tail outputs are DMA'd straight into the AllToAll input DRAM
     buffer at flush time (features -> token shards); collectives trigger
     within ~2us of their heads finishing.  Gathers run on the sync queue,
     emitted late enough to never block it.  Local full-width o_proj per
     core on its token shard, interleaved with batch-1 attention.
Host reassembles the token-sharded outputs.
"""

import math
import sys

for _p in ("/opt/trn_rl_repo",):
    if _p not in sys.path:
        sys.path.insert(0, _p)

import numpy as np
import ml_dtypes

import concourse.bass as bass
import concourse.mybir as mybir
import concourse.tile as tile
from concourse import bacc, bass_utils

F32 = mybir.dt.float32
F32R = mybir.dt.float32r
BF16 = mybir.dt.bfloat16
BF = ml_dtypes.bfloat16


class Cfg:
    def __init__(self, hidden=5120, n_heads=40, dh=128, B=2, S=2048, n_cores=8):
        self.hidden = hidden
        self.n_heads = n_heads
        self.dh = dh
        self.B = B
        self.S = S
        self.n_cores = n_cores
        assert dh == 128
        self.HL = n_heads // n_cores          # heads per core (5)
        self.F = 3 * self.HL * dh             # per-core packed qkv rows (1920)
        self.FO = self.HL * dh                # per-core attn feature width (640)
        self.T = B * S                        # total tokens (4096)
        self.KC = hidden // 128               # contraction chunks (40)
        self.TC = self.T // 512               # token chunks for qkv (8)
        self.SKT = S // 128                   # k tiles per batch seq (16)
        self.QC = S // 512                    # q chunks per batch (4)
        self.NFT = self.F // 128              # qkv feature tiles (15)
        self.TSH = S // n_cores               # token shard per core per batch (256)
        self.OC = hidden // 512               # o_proj out chunks (10)

    def part_heads(self):
        if self.HL > 3:
            return [(0, 3), (3, self.HL)]
        return [(0, self.HL)]

    def key(self):
        return (self.hidden, self.n_heads, self.dh, self.B, self.S, self.n_cores)


def build_program(cfg: Cfg, mode: str):
    """mode: 'causal' (mult-mask diag blocks + block skip), 'dense' (no mask),
    'masked' (general additive mask, host passes maskT pre-scaled)."""
    assert mode in ("causal", "dense", "masked")
    c = cfg
    nc = bacc.Bacc("TRN2", target_bir_lowering=False, debug=False,
                   num_devices=c.n_cores)

    xt = nc.dram_tensor("xt", [c.hidden, c.T], BF16, kind="ExternalInput").ap()
    wqkvt = nc.dram_tensor("wqkvt", [c.hidden, c.F], BF16,
                           kind="ExternalInput").ap()
    wot = nc.dram_tensor("wot", [c.hidden, c.hidden], BF16,
                         kind="ExternalInput").ap()
    mask_ext = None
    if mode == "masked":
        mask_ext = nc.dram_tensor("maskt", [c.S, c.S], F32,
                                  kind="ExternalInput").ap()
    # per-core output: for each batch, this core's token shard (all hidden)
    out_ext = nc.dram_tensor("out", [c.B, c.TSH, c.hidden], F32,
                             kind="ExternalOutput").ap()

    inv_sqrt_dh = 1.0 / math.sqrt(c.dh)

    xt_r = xt.rearrange("(kc p) t -> p kc t", p=128)
    wq_r = wqkvt.rearrange("(kc p) f -> p kc f", p=128)
    wo_r = wot.rearrange("(kc p) j -> p kc j", p=128)

    part_heads = c.part_heads()
    two_parts = len(part_heads) > 1

    def part_of(h):
        for p, (h0, h1) in enumerate(part_heads):
            if h0 <= h < h1:
                return p, h0, h1
        raise AssertionError

    with tile.TileContext(nc) as tc:
        with tc.tile_pool(name="dram", bufs=1, space="DRAM") as dram:
            qkv = dram.tile([2 * c.HL, 128, c.T], BF16)
            vdram = dram.tile([c.B, c.SKT, 128, c.FO], BF16, tag="vdram",
                              name="vdram")
            a2a_in = {}
            a2a_out = {}
            for b in range(c.B):
                for p, (h0, h1) in enumerate(part_heads):
                    nh = h1 - h0
                    a2a_in[(b, p)] = dram.tile(
                        [c.n_cores, nh * 128, c.TSH], BF16,
                        tag=f"a2ai{b}{p}", name=f"a2ai{b}{p}")
                    a2a_out[(b, p)] = dram.tile(
                        [c.n_cores, nh * 128, c.TSH], BF16,
                        tag=f"a2ao{b}{p}", name=f"a2ao{b}{p}")

            # ---------------- Phase A: QKV projection -------------------
            # qkv[ft, d, t] = sum_h W[h, ft*128+d] * X[h, t]  (q,k feature-
            # major); V is produced token-major into vdram inside split 0,
            # reusing the resident xq tiles (no extra X pass).
            splits = [c.HL, c.HL]
            with tc.tile_pool(name="qkv_w", bufs=1) as wpool, \
                 tc.tile_pool(name="av_w", bufs=1) as wvpool, \
                 tc.tile_pool(name="qkv_x", bufs=2) as xpool, \
                 tc.tile_pool(name="qkv_o", bufs=8) as opool, \
                 tc.tile_pool(name="av_o", bufs=2) as ovpool, \
                 tc.tile_pool(name="qkv_ps", bufs=5, space="PSUM") as pspool, \
                 tc.tile_pool(name="av_pa", bufs=2, space="PSUM") as avpa, \
                 tc.tile_pool(name="av_pb", bufs=1, space="PSUM") as avpb:
                assert c.KC % 4 == 0
                KQ = c.KC // 4
                wvs = None
                ft0 = 0
                for si, nft in enumerate(splits):
                    wts = None
                    for tci in range(c.TC):
                        xq = [xpool.tile([128, KQ, 512], BF16, tag=f"x{j}",
                                         name=f"x{j}") for j in range(4)]
                        for j in range(4):
                            if si == 0 and tci == 0:
                                # sliced: the first matmul starts after one
                                # kc-slice lands instead of the whole tile
                                for kq in range(KQ):
                                    nc.sync.dma_start(
                                        xq[j][:, kq, :],
                                        xt_r[:, j * KQ + kq, 0:512])
                            else:
                                nc.sync.dma_start(
                                    xq[j][:],
                                    xt_r[:, j * KQ:(j + 1) * KQ,
                                         tci * 512:(tci + 1) * 512])
                        if tci == 0:
                            # per-kc weight tiles: lets the next group's
                            # weight loads overlap this group's tail
                            wts = []
                            for kc in range(c.KC):
                                w_t = wpool.tile([128, max(splits) * 128],
                                                 BF16, tag=f"w{kc}",
                                                 name=f"w{kc}")
                                nc.sync.dma_start(
                                    w_t[:, :nft * 128],
                                    wq_r[:, kc,
                                         ft0 * 128:(ft0 + nft) * 128])
                                wts.append(w_t)
                        if si == 0 and tci == 0:
                            # V weights, queued behind the first x/w tiles
                            wvs = []
                            for kc in range(c.KC):
                                wv_t = wvpool.tile([128, c.FO], BF16,
                                                   tag=f"wv{kc}",
                                                   name=f"wv{kc}")
                                nc.sync.dma_start(
                                    wv_t[:], wq_r[:, kc, 2 * c.FO:3 * c.FO])
                                wvs.append(wv_t)
                        pss = [pspool.tile([128, 512], F32, tag="ps",
                                           name=f"ps{i}")
                               for i in range(nft)]
                        for kc in range(c.KC):
                            for i in range(nft):
                                nc.tensor.matmul(
                                    pss[i][:],
                                    wts[kc][:, i * 128:(i + 1) * 128],
                                    xq[kc // KQ][:, kc % KQ, :],
                                    start=(kc == 0), stop=(kc == c.KC - 1))
                        for i in range(nft):
                            o_sb = opool.tile([128, 512], BF16, tag="o")
                            nc.vector.tensor_copy(o_sb[:], pss[i][:])
                            nc.sync.dma_start(
                                qkv[ft0 + i, :, tci * 512:(tci + 1) * 512],
                                o_sb[:])
                        if si == 0:
                            # V token-major for this tc's 4 token tiles:
                            # vdram[b, st, tok_p, f] = sum_h X[h,tok] Wv[h,f]
                            for j2 in range(4):
                                tt = tci * 4 + j2
                                vb, st = tt // c.SKT, tt % c.SKT
                                ps_a = avpa.tile([128, 512], F32, tag="pa",
                                                 name="ps_a")
                                ps_b = avpb.tile([128, c.FO - 512], F32,
                                                 tag="pb", name="ps_b")
                                for kc in range(c.KC):
                                    xsl = xq[kc // KQ][:, kc % KQ,
                                              j2 * 128:(j2 + 1) * 128]
                                    nc.tensor.matmul(
                                        ps_a[:], xsl, wvs[kc][:, :512],
                                        start=(kc == 0),
                                        stop=(kc == c.KC - 1))
                                    nc.tensor.matmul(
                                        ps_b[:], xsl, wvs[kc][:, 512:],
                                        start=(kc == 0),
                                        stop=(kc == c.KC - 1))
                                ov = ovpool.tile([128, c.FO], BF16,
                                                 tag="ov", name="ov")
                                nc.vector.tensor_copy(ov[:, :512], ps_a[:])
                                nc.scalar.activation(
                                    ov[:, 512:], ps_b[:],
                                    mybir.ActivationFunctionType.Copy)
                                nc.sync.dma_start(vdram[vb, st], ov[:])
                    ft0 += nft

            # ---------------- Phase B + C (interleaved per batch) -------
            npt = (2 * c.SKT - 3) if mode == "causal" else (2 * c.SKT + 1)
            with tc.tile_pool(name="att_const", bufs=1) as cpool, \
                 tc.tile_pool(name="att_in", bufs=2) as inpool, \
                 tc.tile_pool(name="att_v", bufs=2) as vpool, \
                 tc.tile_pool(name="att_pt", bufs=npt) as ptpool, \
                 tc.tile_pool(name="att_acc", bufs=2) as accpool, \
                 tc.tile_pool(name="att_sm", bufs=2) as smpool, \
                 tc.tile_pool(name="att_fl", bufs=3) as flpool, \
                 tc.tile_pool(name="att_ms", bufs=(4 if mode == "masked" else 1)) as mspool, \
                 tc.tile_pool(name="op_attn", bufs=2) as apool, \
                 tc.tile_pool(name="op_w", bufs=5) as wopool, \
                 tc.tile_pool(name="op_o", bufs=2) as oopool, \
                 tc.tile_pool(name="ps_s", bufs=2, space="PSUM") as ps_s, \
                 tc.tile_pool(name="ps_at", bufs=2, space="PSUM") as ps_at, \
                 tc.tile_pool(name="ps_ms", bufs=2, space="PSUM") as ps_ms, \
                 tc.tile_pool(name="ps_op", bufs=2, space="PSUM") as ps_op:

                # constants
                ones_col_b = cpool.tile([128, 1], BF16)   # lhsT for pt colsum
                ones_col_r = cpool.tile([128, 1], F32R)   # lhsT for acc merge
                ones_row = cpool.tile([1, 128], F32R)     # lhsT for broadcast
                with tc.tile_pool(name="att_tmp", bufs=1) as tmppool:
                    o32 = tmppool.tile([128, 1], F32, tag="o32")
                    nc.vector.memset(o32[:], 1.0)
                    nc.vector.tensor_copy(ones_col_b[:], o32[:])
                    nc.vector.tensor_copy(ones_col_r[:], o32[:])
                    r32 = tmppool.tile([1, 128], F32, tag="r32")
                    nc.vector.memset(r32[:], 1.0)
                    nc.vector.tensor_copy(ones_row[:], r32[:])
                ctri = None
                if mode == "causal":
                    # multiplicative triangle mask [128k, 128q]:
                    # m[p, y] = 1 where y >= p else 0
                    with tc.tile_pool(name="att_cm", bufs=1) as cmtmp:
                        m32 = cmtmp.tile([128, 128], F32, tag="m32")
                        nc.gpsimd.memset(m32[:], 1.0)
                        nc.gpsimd.affine_select(
                            out=m32[:], in_=m32[:],
                            compare_op=mybir.AluOpType.is_ge, fill=0.0,
                            base=0, pattern=[[1, 128]],
                            channel_multiplier=-1)
                        ctri = cpool.tile([128, 128], BF16, tag="ctri")
                        nc.vector.tensor_copy(ctri[:], m32[:])

                pending = []

                def emit_pv(prev, j):
                    # one PV matmul of the deferred (previous) chunk
                    off = j - 4 * prev["qc"]
                    q0 = off * 128 if (mode == "causal" and off > 0) else 0
                    nkt_p = len(prev["pts"])
                    nc.tensor.matmul(
                        prev["at_ps"][:, q0:], prev["v_tok"][:, j, :],
                        prev["pts"][j][:, q0:],
                        start=(j == 0), stop=(j == nkt_p - 1))

                def early_finish(prev):
                    # reciprocal path of the previous chunk's denominator:
                    # [1,512] PSUM -> SBUF on scalar, broadcast via PE,
                    # full-parallel reciprocal on vector
                    den_sb = smpool.tile([1, 512], F32R, tag="densb",
                                         name="den_sb")
                    nc.scalar.activation(den_sb[:], prev["den_ps"][:],
                                         mybir.ActivationFunctionType.Copy)
                    bc_ps = ps_ms.tile([128, 512], F32, tag="ms",
                                       name="bc_ps")
                    nc.tensor.matmul(bc_ps[:], ones_row[:], den_sb[:],
                                     start=True, stop=True)
                    rbc = smpool.tile([128, 512], F32, tag="rbc", name="rbc")
                    nc.vector.reciprocal_approx_fast(rbc[:], bc_ps[:])
                    prev["rbc"] = rbc

                def late_finish(prev):
                    # normalized output chunk + staging DMA into the
                    # AllToAll input buffer (token-shard layout)
                    ofl = flpool.tile([128, 512], BF16, tag="ofl",
                                      name="ofl")
                    nc.vector.tensor_tensor(ofl[:], prev["at_ps"][:],
                                            prev["rbc"][:],
                                            mybir.AluOpType.mult)
                    p, h0, _ = part_of(prev["h"])
                    qc_p = prev["qc"]
                    dst = a2a_in[(prev["b"], p)].rearrange(
                        "s (f q) t -> q s f t", q=128)[
                        :, 2 * qc_p:2 * qc_p + 2, prev["h"] - h0, :]
                    nc.sync.dma_start(
                        dst, ofl[:].rearrange("q (s t) -> q s t", s=2))

                def flush_serial():
                    if not pending:
                        return
                    prev = pending.pop()
                    at_ps = ps_at.tile([128, 512], F32, tag="at",
                                       name="at_ps")
                    prev["at_ps"] = at_ps
                    early_finish(prev)
                    for j in range(len(prev["pts"])):
                        emit_pv(prev, j)
                    late_finish(prev)

                def attend_head(b, h):
                    t0 = b * c.S
                    q_sb = inpool.tile([128, c.S], BF16, tag="q",
                                       name="q_sb")
                    k_sb = inpool.tile([128, c.S], BF16, tag="k",
                                       name="k_sb")
                    # sliced loads for the first head: the first score
                    # matmul starts after one 512-token slice lands
                    nslc = 4 if (b == 0 and h == 0) else 1
                    for sl in range(nslc):
                        w = c.S // nslc
                        nc.sync.dma_start(
                            q_sb[:, sl * w:(sl + 1) * w],
                            qkv[h, :, t0 + sl * w:t0 + (sl + 1) * w])
                        nc.sync.dma_start(
                            k_sb[:, sl * w:(sl + 1) * w],
                            qkv[c.HL + h, :, t0 + sl * w:t0 + (sl + 1) * w])
                    v_tok = vpool.tile([128, c.SKT, 128], BF16, tag="vt",
                                       name="v_tok")
                    nc.sync.dma_start(
                        v_tok[:],
                        vdram[b].rearrange("s p d -> p s d")[
                            :, :, h * 128:(h + 1) * 128])

                    for qc in range(c.QC):
                        nkt = 4 * (qc + 1) if mode == "causal" else c.SKT
                        prev = pending.pop() if pending else None
                        if prev is not None:
                            at_ps = ps_at.tile([128, 512], F32, tag="at",
                                               name="at_ps")
                            prev["at_ps"] = at_ps
                        den_ps = ps_ms.tile([1, 512], F32, tag="ms",
                                            name="den_ps")
                        acc_v = accpool.tile([128, 512], F32R, tag="accv",
                                             name="acc_v")
                        pnk = len(prev["pts"]) if prev is not None else 0
                        emitted = 0
                        first_pe = True
                        pts = []
                        for kt in range(nkt):
                            off = kt - 4 * qc  # >=0: diagonal tile (causal)
                            pt = ptpool.tile([128, 512], BF16, tag="pt",
                                             name="pt")
                            s_ps = ps_s.tile([128, 512], F32, tag="s",
                                             name="s_ps")
                            if mode == "causal" and off > 0:
                                # valid q range is [off*128, 512); the
                                # region below is never read (trimmed)
                                q0 = off * 128
                                w = 512 - q0
                                nc.tensor.matmul(
                                    s_ps[:, :w],
                                    k_sb[:, kt * 128:(kt + 1) * 128],
                                    q_sb[:, qc * 512 + q0:(qc + 1) * 512],
                                    start=True, stop=True)
                                nc.scalar.activation(
                                    pt[:, q0:], s_ps[:, :w],
                                    mybir.ActivationFunctionType.Exp,
                                    scale=inv_sqrt_dh)
                                nc.vector.tensor_tensor(
                                    pt[:, q0:q0 + 128],
                                    pt[:, q0:q0 + 128],
                                    ctri[:], mybir.AluOpType.mult)
                            else:
                                q0 = 0
                                nc.tensor.matmul(
                                    s_ps[:],
                                    k_sb[:, kt * 128:(kt + 1) * 128],
                                    q_sb[:, qc * 512:(qc + 1) * 512],
                                    start=True, stop=True)
                                if mode == "masked":
                                    m_sb = mspool.tile([128, 512], F32,
                                                       tag="m", name="m_sb")
                                    nc.sync.dma_start(
                                        m_sb[:],
                                        mask_ext[kt * 128:(kt + 1) * 128,
                                                 qc * 512:(qc + 1) * 512])
                                    nc.vector.tensor_tensor(
                                        s_ps[:], s_ps[:], m_sb[:],
                                        mybir.AluOpType.add)
                                nc.scalar.activation(
                                    pt[:], s_ps[:],
                                    mybir.ActivationFunctionType.Exp,
                                    scale=inv_sqrt_dh)
                                if mode == "causal" and off == 0:
                                    nc.vector.tensor_tensor(
                                        pt[:, :128], pt[:, :128],
                                        ctri[:], mybir.AluOpType.mult)
                            # denominator: even tiles on a vector chain,
                            # odd tiles as PE colsum matmuls
                            if kt % 2 == 0:
                                if kt == 0:
                                    nc.vector.tensor_copy(acc_v[:], pt[:])
                                else:
                                    nc.vector.tensor_tensor(
                                        acc_v[:, q0:], acc_v[:, q0:],
                                        pt[:, q0:], mybir.AluOpType.add)
                            else:
                                nc.tensor.matmul(
                                    den_ps[:, q0:], ones_col_b[:],
                                    pt[:, q0:],
                                    start=first_pe, stop=False)
                                first_pe = False
                            pts.append(pt)
                            # interleave the deferred PV matmuls of the
                            # previous chunk between this chunk's score
                            # matmuls: PE stays busy while exp() drains
                            if prev is not None:
                                tgt = ((kt + 1) * pnk) // nkt
                                while emitted < tgt:
                                    emit_pv(prev, emitted)
                                    emitted += 1
                                if kt == 0:
                                    early_finish(prev)
                        # fold the vector chain into the denominator
                        nc.tensor.matmul(den_ps[:], ones_col_r[:], acc_v[:],
                                         start=first_pe, stop=True)
                        if prev is not None:
                            late_finish(prev)
                        pending.append(dict(
                            b=b, h=h, qc=qc, den_ps=den_ps, pts=pts,
                            v_tok=v_tok))

                def launch_a2a(b, p):
                    flush_serial()
                    nc.gpsimd.collective_compute(
                        "AllToAll",
                        mybir.AluOpType.bypass,
                        replica_groups=[list(range(c.n_cores))],
                        ins=[a2a_in[(b, p)][:].opt()],
                        outs=[a2a_out[(b, p)][:].opt()],
                    )

                def gather(b, p):
                    h0, h1 = part_heads[p]
                    nh = h1 - h0
                    attn_sb = apool.tile([128, c.n_cores * nh, c.TSH], BF16,
                                         tag=f"ag{p}", name=f"ag{b}{p}")
                    nc.sync.dma_start(
                        attn_sb[:],
                        a2a_out[(b, p)].rearrange("s (f q) t -> q (s f) t",
                                                  q=128))
                    return attn_sb

                # per-oc contraction layout: (part, fc offset in part, count)
                wo_layout = []
                fc0 = 0
                for p, (h0, h1) in enumerate(part_heads):
                    nfc = c.n_cores * (h1 - h0)
                    ka = nfc // 2
                    wo_layout.append((p, 0, ka, fc0))
                    wo_layout.append((p, ka, nfc - ka, fc0 + ka))
                    fc0 += nfc

                max_cnt = max(le[2] for le in wo_layout)

                def load_wo(oc):
                    wo_sbs = []
                    for (p, k0, cnt, gfc) in wo_layout:
                        wo_t = wopool.tile([128, max_cnt, 512], BF16,
                                           tag="wo", name="wo_t")
                        nc.sync.dma_start(
                            wo_t[:, :cnt, :],
                            wo_r[:, gfc:gfc + cnt,
                                 oc * 512:(oc + 1) * 512])
                        wo_sbs.append(wo_t)
                    return wo_sbs

                def o_proj_chunk(b, parts, oc, wo_sbs=None):
                    if wo_sbs is None:
                        wo_sbs = load_wo(oc)
                    last = len(wo_layout) - 1
                    for tt in range(c.TSH // 128):
                        ps = ps_op.tile([128, 512], F32, tag="ops",
                                        name="op_ps")
                        for wi, (p, k0, cnt, gfc) in enumerate(wo_layout):
                            for k in range(cnt):
                                nc.tensor.matmul(
                                    ps[:],
                                    parts[p][:, k0 + k,
                                             tt * 128:(tt + 1) * 128],
                                    wo_sbs[wi][:, k, :],
                                    start=(wi == 0 and k == 0),
                                    stop=(wi == last and k == cnt - 1))
                        po_sb = oopool.tile([128, 512], F32, tag="po",
                                            name="po_sb")
                        nc.vector.tensor_copy(po_sb[:], ps[:])
                        nc.gpsimd.dma_start(
                            out_ext[b, tt * 128:(tt + 1) * 128,
                                    oc * 512:(oc + 1) * 512],
                            po_sb[:])

                # ---- schedule ----
                h_p0 = part_heads[0][1] - 1
                # batch 0 attention; part collectives trigger right after
                # their last head's tail is flushed
                for h in range(c.HL):
                    attend_head(0, h)
                    if two_parts and h == h_p0:
                        launch_a2a(0, 0)
                launch_a2a(0, 1 if two_parts else 0)
                # batch 1 attention interleaved with batch-0 o_proj
                attend_head(1, 0)
                g00 = gather(0, 0)
                attend_head(1, 1)
                parts0 = [g00, gather(0, 1)] if two_parts else [g00]
                attend_head(1, 2)
                if two_parts:
                    launch_a2a(1, 0)
                attend_head(1, 3)
                o_proj_chunk(0, parts0, 0)
                o_proj_chunk(0, parts0, 1)
                attend_head(1, 4)
                if two_parts:
                    launch_a2a(1, 1)
                for oc in range(2, c.OC):
                    o_proj_chunk(0, parts0, oc)
                if two_parts:
                    g10 = gather(1, 0)
                parts1 = [g10, gather(1, 1)] if two_parts else [gather(1, 0)]
                wo_next = load_wo(0)
                for oc in range(c.OC):
                    wo_cur = wo_next
                    wo_next = load_wo(oc + 1) if oc + 1 < c.OC else None
                    o_proj_chunk(1, parts1, oc, wo_cur)

    nc.compile()
    return nc


# --------------------------------------------------------------------------
_CACHE = {}


def _get_program(cfg: Cfg, mode: str):
    key = (cfg.key(), mode)
    if key not in _CACHE:
        _CACHE[key] = build_program(cfg, mode)
    return _CACHE[key]


def prepare_inputs(cfg: Cfg, hidden_states, attention_mask, W_pack, W_o):
    """Host-side shard + layout prep (bf16 cast). Returns (mode, in_maps)."""
    c = cfg
    X = np.asarray(hidden_states, dtype=np.float32).reshape(c.T, c.hidden)
    XT = np.ascontiguousarray(X.T).astype(BF)

    mask = np.asarray(attention_mask, dtype=np.float32).reshape(c.S, c.S)
    causal_ref = np.where(
        np.tril(np.ones((c.S, c.S), dtype=bool)), 0.0, -1e9
    ).astype(np.float32)
    if np.array_equal(mask, causal_ref):
        mode = "causal"
    elif not mask.any():
        mode = "dense"
    else:
        mode = "masked"

    W_pack = np.asarray(W_pack, dtype=np.float32)
    W_o = np.asarray(W_o, dtype=np.float32)
    H = c.hidden
    # woT rows (features) reordered to the part-concatenated gather order:
    # for each head part, src-core-major then local head
    order = [s * c.HL + j
             for (h0, h1) in c.part_heads()
             for s in range(c.n_cores)
             for j in range(h0, h1)]
    woT = np.ascontiguousarray(
        W_o.T.reshape(c.n_heads, c.dh, c.hidden)[order]
        .reshape(c.hidden, c.hidden)).astype(BF)   # [feat, out] full
    in_maps = []
    for g in range(c.n_cores):
        r0, r1 = g * c.FO, (g + 1) * c.FO
        wq = W_pack[r0:r1]
        wk = W_pack[H + r0:H + r1]
        wv = W_pack[2 * H + r0:2 * H + r1]
        wqkvT = np.ascontiguousarray(
            np.concatenate([wq, wk, wv], axis=0).T).astype(BF)  # [H, F]
        m = {"xt": XT, "wqkvt": wqkvT, "wot": woT}
        if mode == "masked":
            m["maskt"] = np.ascontiguousarray(mask.T * math.sqrt(c.dh))
        in_maps.append(m)
    return mode, in_maps


def assemble_output(cfg: Cfg, results):
    c = cfg
    full = np.empty((c.B, c.S, c.hidden), dtype=np.float32)
    for g in range(c.n_cores):
        o = results[g]["out"].reshape(c.B, c.TSH, c.hidden)
        for b in range(c.B):
            full[b, g * c.TSH:(g + 1) * c.TSH] = o[b]
    return full


def kernel(hidden_states, attention_mask, W_pack, W_o):
    cfg = Cfg()
    mode, in_maps = prepare_inputs(cfg, hidden_states, attention_mask,
                                   W_pack, W_o)
    nc = _get_program(cfg, mode)
    res = bass_utils.run_bass_kernel_spmd(nc, in_maps,
                                          list(range(cfg.n_cores)))
    return assemble_output(cfg, res.results)


# revision 12
# speedup vs baseline: 1.0136x; 1.0136x over previous
"""Trainium2 Bass kernel for BaichuanAttention (hidden=5120, 40 heads, b=2, s=2048).

Tensor-parallel over heads across 8 NeuronCores, bf16 compute:
  A) QKV projection with SBUF-resident bf16 weights, X streamed.
  B) Flash-style causal attention in S^T form (scores computed as K^T.Q so
     exp() writes P^T directly -- no P transposes), V transposed on-chip.
     Score matmuls of chunk qc are interleaved with the PV matmuls of chunk
     qc-1 in PE program order so the exp() latency never gates the PE.
  C) Softmax-tail outputs are DMA'd straight into the AllToAll input DRAM
     buffer at flush time (features -> token shards); collectives trigger
     within ~2us of their heads finishing.  Gathers run on the sync queue,
     emitted late enough to never block it.  Local full-width o_proj per
     core on its token shard, interleaved with batch-1 attention.
Host reassembles the token-sharded outputs.
"""

import math
import sys

for _p in ("/opt/trn_rl_repo",):
    if _p not in sys.path:
        sys.path.insert(0, _p)

import numpy as np
import ml_dtypes

import concourse.bass as bass
import concourse.mybir as mybir
import concourse.tile as tile
from concourse import bacc, bass_utils

F32 = mybir.dt.float32
F32R = mybir.dt.float32r
BF16 = mybir.dt.bfloat16
BF = ml_dtypes.bfloat16


class Cfg:
    def __init__(self, hidden=5120, n_heads=40, dh=128, B=2, S=2048, n_cores=8):
        self.hidden = hidden
        self.n_heads = n_heads
        self.dh = dh
        self.B = B
        self.S = S
        self.n_cores = n_cores
        assert dh == 128
        self.HL = n_heads // n_cores          # heads per core (5)
        self.F = 3 * self.HL * dh             # per-core packed qkv rows (1920)
        self.FO = self.HL * dh                # per-core attn feature width (640)
        self.T = B * S                        # total tokens (4096)
        self.KC = hidden // 128               # contraction chunks (40)
        self.TC = self.T // 512               # token chunks for qkv (8)
        self.SKT = S // 128                   # k tiles per batch seq (16)
        self.QC = S // 512                    # q chunks per batch (4)
        self.NFT = self.F // 128              # qkv feature tiles (15)
        self.TSH = S // n_cores               # token shard per core per batch (256)
        self.OC = hidden // 512               # o_proj out chunks (10)

    def part_heads(self):
        if self.HL > 3:
            return [(0, 3), (3, self.HL)]
        return [(0, self.HL)]

    def key(self):
        return (self.hidden, self.n_heads, self.dh, self.B, self.S, self.n_cores)


def build_program(cfg: Cfg, mode: str):
    """mode: 'causal' (mult-mask diag blocks + block skip), 'dense' (no mask),
    'masked' (general additive mask, host passes maskT pre-scaled)."""
    assert mode in ("causal", "dense", "masked")
    c = cfg
    nc = bacc.Bacc("TRN2", target_bir_lowering=False, debug=False,
                   num_devices=c.n_cores)

    xt = nc.dram_tensor("xt", [c.hidden, c.T], BF16, kind="ExternalInput").ap()
    wqkvt = nc.dram_tensor("wqkvt", [c.hidden, c.F], BF16,
                           kind="ExternalInput").ap()
    wot = nc.dram_tensor("wot", [c.hidden, c.hidden], BF16,
                         kind="ExternalInput").ap()
    mask_ext = None
    if mode == "masked":
        mask_ext = nc.dram_tensor("maskt", [c.S, c.S], F32,
                                  kind="ExternalInput").ap()
    # per-core output: for each batch, this core's token shard (all hidden)
    out_ext = nc.dram_tensor("out", [c.B, c.TSH, c.hidden], F32,
                             kind="ExternalOutput").ap()

    inv_sqrt_dh = 1.0 / math.sqrt(c.dh)

    xt_r = xt.rearrange("(kc p) t -> p kc t", p=128)
    wq_r = wqkvt.rearrange("(kc p) f -> p kc f", p=128)
    wo_r = wot.rearrange("(kc p) j -> p kc j", p=128)

    part_heads = c.part_heads()
    two_parts = len(part_heads) > 1

    def part_of(h):
        for p, (h0, h1) in enumerate(part_heads):
            if h0 <= h < h1:
                return p, h0, h1
        raise AssertionError

    with tile.TileContext(nc) as tc:
        with tc.tile_pool(name="dram", bufs=1, space="DRAM") as dram:
            qkv = dram.tile([2 * c.HL, 128, c.T], BF16)
            vdram = dram.tile([c.B, c.SKT, 128, c.FO], BF16, tag="vdram",
                              name="vdram")
            a2a_in = {}
            a2a_out = {}
            for b in range(c.B):
                for p, (h0, h1) in enumerate(part_heads):
                    nh = h1 - h0
                    a2a_in[(b, p)] = dram.tile(
                        [c.n_cores, nh * 128, c.TSH], BF16,
                        tag=f"a2ai{b}{p}", name=f"a2ai{b}{p}")
                    a2a_out[(b, p)] = dram.tile(
                        [c.n_cores, nh * 128, c.TSH], BF16,
                        tag=f"a2ao{b}{p}", name=f"a2ao{b}{p}")

            # ---------------- Phase A: QKV projection -------------------
            # qkv[ft, d, t] = sum_h W[h, ft*128+d] * X[h, t]  (q,k feature-
            # major); V is produced token-major into vdram inside split 0,
            # reusing the resident xq tiles (no extra X pass).
            splits = [c.HL, c.HL]
            with tc.tile_pool(name="qkv_w", bufs=1) as wpool, \
                 tc.tile_pool(name="av_w", bufs=1) as wvpool, \
                 tc.tile_pool(name="qkv_x", bufs=2) as xpool, \
                 tc.tile_pool(name="qkv_o", bufs=8) as opool, \
                 tc.tile_pool(name="av_o", bufs=2) as ovpool, \
                 tc.tile_pool(name="qkv_ps", bufs=5, space="PSUM") as pspool, \
                 tc.tile_pool(name="av_pa", bufs=2, space="PSUM") as avpa, \
                 tc.tile_pool(name="av_pb", bufs=1, space="PSUM") as avpb:
                assert c.KC % 4 == 0
                KQ = c.KC // 4
                wvs = None
                ft0 = 0
                for si, nft in enumerate(splits):
                    wts = None
                    for tci in range(c.TC):
                        xq = [xpool.tile([128, KQ, 512], BF16, tag=f"x{j}",
                                         name=f"x{j}") for j in range(4)]
                        for j in range(4):
                            if si == 0 and tci == 0:
                                # sliced: the first matmul starts after one
                                # kc-slice lands instead of the whole tile
                                for kq in range(KQ):
                                    nc.sync.dma_start(
                                        xq[j][:, kq, :],
                                        xt_r[:, j * KQ + kq, 0:512])
                            else:
                                nc.sync.dma_start(
                                    xq[j][:],
                                    xt_r[:, j * KQ:(j + 1) * KQ,
                                         tci * 512:(tci + 1) * 512])
                        if tci == 0:
                            # per-kc weight tiles: lets the next group's
                            # weight loads overlap this group's tail
                            wts = []
                            for kc in range(c.KC):
                                w_t = wpool.tile([128, max(splits) * 128],
                                                 BF16, tag=f"w{kc}",
                                                 name=f"w{kc}")
                                nc.sync.dma_start(
                                    w_t[:, :nft * 128],
                                    wq_r[:, kc,
                                         ft0 * 128:(ft0 + nft) * 128])
                                wts.append(w_t)
                        if si == 0 and tci == 0:
                            # V weights, queued behind the first x/w tiles
                            wvs = []
                            for kc in range(c.KC):
                                wv_t = wvpool.tile([128, c.FO], BF16,
                                                   tag=f"wv{kc}",
                                                   name=f"wv{kc}")
                                nc.sync.dma_start(
                                    wv_t[:], wq_r[:, kc, 2 * c.FO:3 * c.FO])
                                wvs.append(wv_t)
                        pss = [pspool.tile([128, 512], F32, tag="ps",
                                           name=f"ps{i}")
                               for i in range(nft)]
                        for kc in range(c.KC):
                            for i in range(nft):
                                nc.tensor.matmul(
                                    pss[i][:],
                                    wts[kc][:, i * 128:(i + 1) * 128],
                                    xq[kc // KQ][:, kc % KQ, :],
                                    start=(kc == 0), stop=(kc == c.KC - 1))
                        for i in range(nft):
                            o_sb = opool.tile([128, 512], BF16, tag="o")
                            nc.vector.tensor_copy(o_sb[:], pss[i][:])
                            nc.sync.dma_start(
                                qkv[ft0 + i, :, tci * 512:(tci + 1) * 512],
                                o_sb[:])
                        if si == 0:
                            # V token-major for this tc's 4 token tiles:
                            # vdram[b, st, tok_p, f] = sum_h X[h,tok] Wv[h,f]
                            for j2 in range(4):
                                tt = tci * 4 + j2
                                vb, st = tt // c.SKT, tt % c.SKT
                                ps_a = avpa.tile([128, 512], F32, tag="pa",
                                                 name="ps_a")
                                ps_b = avpb.tile([128, c.FO - 512], F32,
                                                 tag="pb", name="ps_b")
                                for kc in range(c.KC):
                                    xsl = xq[kc // KQ][:, kc % KQ,
                                              j2 * 128:(j2 + 1) * 128]
                                    nc.tensor.matmul(
                                        ps_a[:], xsl, wvs[kc][:, :512],
                                        start=(kc == 0),
                                        stop=(kc == c.KC - 1))
                                    nc.tensor.matmul(
                                        ps_b[:], xsl, wvs[kc][:, 512:],
                                        start=(kc == 0),
                                        stop=(kc == c.KC - 1))
                                ov = ovpool.tile([128, c.FO], BF16,
                                                 tag="ov", name="ov")
                                nc.vector.tensor_copy(ov[:, :512], ps_a[:])
                                nc.scalar.activation(
                                    ov[:, 512:], ps_b[:],
                                    mybir.ActivationFunctionType.Copy)
                                nc.sync.dma_start(vdram[vb, st], ov[:])
                    ft0 += nft

            # ---------------- Phase B + C (interleaved per batch) -------
            npt = (2 * c.SKT - 3) if mode == "causal" else (2 * c.SKT + 1)
            with tc.tile_pool(name="att_const", bufs=1) as cpool, \
                 tc.tile_pool(name="att_in", bufs=2) as inpool, \
                 tc.tile_pool(name="att_v", bufs=2) as vpool, \
                 tc.tile_pool(name="att_pt", bufs=npt) as ptpool, \
                 tc.tile_pool(name="att_acc", bufs=2) as accpool, \
                 tc.tile_pool(name="att_sm", bufs=2) as smpool, \
                 tc.tile_pool(name="att_fl", bufs=3) as flpool, \
                 tc.tile_pool(name="att_ms", bufs=(4 if mode == "masked" else 1)) as mspool, \
                 tc.tile_pool(name="op_attn", bufs=2) as apool, \
                 tc.tile_pool(name="op_w", bufs=5) as wopool, \
                 tc.tile_pool(name="op_o", bufs=2) as oopool, \
                 tc.tile_pool(name="ps_s", bufs=3, space="PSUM") as ps_s, \
                 tc.tile_pool(name="ps_at", bufs=2, space="PSUM") as ps_at, \
                 tc.tile_pool(name="ps_ms", bufs=1, space="PSUM") as ps_ms, \
                 tc.tile_pool(name="ps_op", bufs=2, space="PSUM") as ps_op:

                # constants
                ones_col_b = cpool.tile([128, 1], BF16)   # lhsT for pt colsum
                ones_col_r = cpool.tile([128, 1], F32R)   # lhsT for acc merge
                ones_row = cpool.tile([1, 128], F32R)     # lhsT for broadcast
                with tc.tile_pool(name="att_tmp", bufs=1) as tmppool:
                    o32 = tmppool.tile([128, 1], F32, tag="o32")
                    nc.vector.memset(o32[:], 1.0)
                    nc.vector.tensor_copy(ones_col_b[:], o32[:])
                    nc.vector.tensor_copy(ones_col_r[:], o32[:])
                    r32 = tmppool.tile([1, 128], F32, tag="r32")
                    nc.vector.memset(r32[:], 1.0)
                    nc.vector.tensor_copy(ones_row[:], r32[:])
                ctri = None
                if mode == "causal":
                    # multiplicative triangle mask [128k, 128q]:
                    # m[p, y] = 1 where y >= p else 0
                    with tc.tile_pool(name="att_cm", bufs=1) as cmtmp:
                        m32 = cmtmp.tile([128, 128], F32, tag="m32")
                        nc.gpsimd.memset(m32[:], 1.0)
                        nc.gpsimd.affine_select(
                            out=m32[:], in_=m32[:],
                            compare_op=mybir.AluOpType.is_ge, fill=0.0,
                            base=0, pattern=[[1, 128]],
                            channel_multiplier=-1)
                        ctri = cpool.tile([128, 128], BF16, tag="ctri")
                        nc.vector.tensor_copy(ctri[:], m32[:])

                pending = []

                def emit_pv(prev, j):
                    # one PV matmul of the deferred (previous) chunk
                    off = j - 4 * prev["qc"]
                    q0 = off * 128 if (mode == "causal" and off > 0) else 0
                    nkt_p = len(prev["pts"])
                    nc.tensor.matmul(
                        prev["at_ps"][:, q0:], prev["v_tok"][:, j, :],
                        prev["pts"][j][:, q0:],
                        start=(j == 0), stop=(j == nkt_p - 1))

                def early_finish(prev):
                    # reciprocal path of the previous chunk's denominator:
                    # [1,512] PSUM -> SBUF on scalar, broadcast via PE,
                    # full-parallel reciprocal on vector
                    den_sb = smpool.tile([1, 512], F32R, tag="densb",
                                         name="den_sb")
                    nc.scalar.activation(den_sb[:], prev["den_ps"][:],
                                         mybir.ActivationFunctionType.Copy)
                    bc_ps = ps_ms.tile([128, 512], F32, tag="ms",
                                       name="bc_ps")
                    nc.tensor.matmul(bc_ps[:], ones_row[:], den_sb[:],
                                     start=True, stop=True)
                    rbc = smpool.tile([128, 512], F32, tag="rbc", name="rbc")
                    nc.vector.reciprocal_approx_fast(rbc[:], bc_ps[:])
                    prev["rbc"] = rbc

                def late_finish(prev):
                    # normalized output chunk + staging DMA into the
                    # AllToAll input buffer (token-shard layout)
                    ofl = flpool.tile([128, 512], BF16, tag="ofl",
                                      name="ofl")
                    nc.vector.tensor_tensor(ofl[:], prev["at_ps"][:],
                                            prev["rbc"][:],
                                            mybir.AluOpType.mult)
                    p, h0, _ = part_of(prev["h"])
                    qc_p = prev["qc"]
                    dst = a2a_in[(prev["b"], p)].rearrange(
                        "s (f q) t -> q s f t", q=128)[
                        :, 2 * qc_p:2 * qc_p + 2, prev["h"] - h0, :]
                    nc.sync.dma_start(
                        dst, ofl[:].rearrange("q (s t) -> q s t", s=2))

                def flush_serial():
                    if not pending:
                        return
                    prev = pending.pop()
                    at_ps = ps_at.tile([128, 512], F32, tag="at",
                                       name="at_ps")
                    prev["at_ps"] = at_ps
                    early_finish(prev)
                    for j in range(len(prev["pts"])):
                        emit_pv(prev, j)
                    late_finish(prev)

                def attend_head(b, h):
                    t0 = b * c.S
                    q_sb = inpool.tile([128, c.S], BF16, tag="q",
                                       name="q_sb")
                    k_sb = inpool.tile([128, c.S], BF16, tag="k",
                                       name="k_sb")
                    nc.sync.dma_start(q_sb[:], qkv[h, :, t0:t0 + c.S])
                    nc.sync.dma_start(k_sb[:], qkv[c.HL + h, :, t0:t0 + c.S])
                    v_tok = vpool.tile([128, c.SKT, 128], BF16, tag="vt",
                                       name="v_tok")
                    nc.sync.dma_start(
                        v_tok[:],
                        vdram[b].rearrange("s p d -> p s d")[
                            :, :, h * 128:(h + 1) * 128])

                    for qc in range(c.QC):
                        nkt = 4 * (qc + 1) if mode == "causal" else c.SKT
                        prev = pending.pop() if pending else None
                        if prev is not None:
                            at_ps = ps_at.tile([128, 512], F32, tag="at",
                                               name="at_ps")
                            prev["at_ps"] = at_ps
                        den_ps = ps_ms.tile([1, 512], F32, tag="ms",
                                            name="den_ps")
                        acc_v = accpool.tile([128, 512], F32R, tag="accv",
                                             name="acc_v")
                        pnk = len(prev["pts"]) if prev is not None else 0
                        emitted = 0
                        first_pe = True
                        pts = []
                        for kt in range(nkt):
                            off = kt - 4 * qc  # >=0: diagonal tile (causal)
                            pt = ptpool.tile([128, 512], BF16, tag="pt",
                                             name="pt")
                            s_ps = ps_s.tile([128, 512], F32, tag="s",
                                             name="s_ps")
                            if mode == "causal" and off > 0:
                                # valid q range is [off*128, 512); the
                                # region below is never read (trimmed)
                                q0 = off * 128
                                w = 512 - q0
                                nc.tensor.matmul(
                                    s_ps[:, :w],
                                    k_sb[:, kt * 128:(kt + 1) * 128],
                                    q_sb[:, qc * 512 + q0:(qc + 1) * 512],
                                    start=True, stop=True)
                                nc.scalar.activation(
                                    pt[:, q0:], s_ps[:, :w],
                                    mybir.ActivationFunctionType.Exp,
                                    scale=inv_sqrt_dh)
                                nc.vector.tensor_tensor(
                                    pt[:, q0:q0 + 128],
                                    pt[:, q0:q0 + 128],
                                    ctri[:], mybir.AluOpType.mult)
                            else:
                                q0 = 0
                                nc.tensor.matmul(
                                    s_ps[:],
                                    k_sb[:, kt * 128:(kt + 1) * 128],
                                    q_sb[:, qc * 512:(qc + 1) * 512],
                                    start=True, stop=True)
                                if mode == "masked":
                                    m_sb = mspool.tile([128, 512], F32,
                                                       tag="m", name="m_sb")
                                    nc.sync.dma_start(
                                        m_sb[:],
                                        mask_ext[kt * 128:(kt + 1) * 128,
                                                 qc * 512:(qc + 1) * 512])
                                    nc.vector.tensor_tensor(
                                        s_ps[:], s_ps[:], m_sb[:],
                                        mybir.AluOpType.add)
                                nc.scalar.activation(
                                    pt[:], s_ps[:],
                                    mybir.ActivationFunctionType.Exp,
                                    scale=inv_sqrt_dh)
                                if mode == "causal" and off == 0:
                                    nc.vector.tensor_tensor(
                                        pt[:, :128], pt[:, :128],
                                        ctri[:], mybir.AluOpType.mult)
                            # denominator: even tiles on a vector chain,
                            # odd tiles as PE colsum matmuls
                            if kt % 2 == 0:
                                if kt == 0:
                                    nc.vector.tensor_copy(acc_v[:], pt[:])
                                else:
                                    nc.vector.tensor_tensor(
                                        acc_v[:, q0:], acc_v[:, q0:],
                                        pt[:, q0:], mybir.AluOpType.add)
                            else:
                                nc.tensor.matmul(
                                    den_ps[:, q0:], ones_col_b[:],
                                    pt[:, q0:],
                                    start=first_pe, stop=False)
                                first_pe = False
                            pts.append(pt)
                            # interleave the deferred PV matmuls of the
                            # previous chunk between this chunk's score
                            # matmuls: PE stays busy while exp() drains
                            if prev is not None:
                                tgt = ((kt + 1) * pnk) // nkt
                                while emitted < tgt:
                                    emit_pv(prev, emitted)
                                    emitted += 1
                                if kt == 0:
                                    early_finish(prev)
                        # fold the vector chain into the denominator
                        nc.tensor.matmul(den_ps[:], ones_col_r[:], acc_v[:],
                                         start=first_pe, stop=True)
                        if prev is not None:
                            late_finish(prev)
                        pending.append(dict(
                            b=b, h=h, qc=qc, den_ps=den_ps, pts=pts,
                            v_tok=v_tok))

                def launch_a2a(b, p):
                    flush_serial()
                    nc.gpsimd.collective_compute(
                        "AllToAll",
                        mybir.AluOpType.bypass,
                        replica_groups=[list(range(c.n_cores))],
                        ins=[a2a_in[(b, p)][:].opt()],
                        outs=[a2a_out[(b, p)][:].opt()],
                    )

                def gather(b, p):
                    h0, h1 = part_heads[p]
                    nh = h1 - h0
                    attn_sb = apool.tile([128, c.n_cores * nh, c.TSH], BF16,
                                         tag=f"ag{p}", name=f"ag{b}{p}")
                    nc.sync.dma_start(
                        attn_sb[:],
                        a2a_out[(b, p)].rearrange("s (f q) t -> q (s f) t",
                                                  q=128))
                    return attn_sb

                # per-oc contraction layout: (part, fc offset in part, count)
                wo_layout = []
                fc0 = 0
                for p, (h0, h1) in enumerate(part_heads):
                    nfc = c.n_cores * (h1 - h0)
                    ka = nfc // 2
                    wo_layout.append((p, 0, ka, fc0))
                    wo_layout.append((p, ka, nfc - ka, fc0 + ka))
                    fc0 += nfc

                max_cnt = max(le[2] for le in wo_layout)

                def o_proj_chunk(b, parts, oc):
                    wo_sbs = []
                    for (p, k0, cnt, gfc) in wo_layout:
                        wo_t = wopool.tile([128, max_cnt, 512], BF16,
                                           tag="wo", name="wo_t")
                        nc.sync.dma_start(
                            wo_t[:, :cnt, :],
                            wo_r[:, gfc:gfc + cnt,
                                 oc * 512:(oc + 1) * 512])
                        wo_sbs.append(wo_t)
                    last = len(wo_layout) - 1
                    for tt in range(c.TSH // 128):
                        ps = ps_op.tile([128, 512], F32, tag="ops",
                                        name="op_ps")
                        for wi, (p, k0, cnt, gfc) in enumerate(wo_layout):
                            for k in range(cnt):
                                nc.tensor.matmul(
                                    ps[:],
                                    parts[p][:, k0 + k,
                                             tt * 128:(tt + 1) * 128],
                                    wo_sbs[wi][:, k, :],
                                    start=(wi == 0 and k == 0),
                                    stop=(wi == last and k == cnt - 1))
                        po_sb = oopool.tile([128, 512], F32, tag="po",
                                            name="po_sb")
                        nc.vector.tensor_copy(po_sb[:], ps[:])
                        nc.gpsimd.dma_start(
                            out_ext[b, tt * 128:(tt + 1) * 128,
                                    oc * 512:(oc + 1) * 512],
                            po_sb[:])

                # ---- schedule ----
                h_p0 = part_heads[0][1] - 1
                # batch 0 attention; part collectives trigger right after
                # their last head's tail is flushed
                for h in range(c.HL):
                    attend_head(0, h)
                    if two_parts and h == h_p0:
                        launch_a2a(0, 0)
                launch_a2a(0, 1 if two_parts else 0)
                # batch 1 attention interleaved with batch-0 o_proj
                attend_head(1, 0)
                g00 = gather(0, 0)
                attend_head(1, 1)
                parts0 = [g00, gather(0, 1)] if two_parts else [g00]
                attend_head(1, 2)
                if two_parts:
                    launch_a2a(1, 0)
                attend_head(1, 3)
                o_proj_chunk(0, parts0, 0)
                o_proj_chunk(0, parts0, 1)
                attend_head(1, 4)
                if two_parts:
                    launch_a2a(1, 1)
                for oc in range(2, c.OC):
                    o_proj_chunk(0, parts0, oc)
                if two_parts:
                    g10 = gather(1, 0)
                parts1 = [g10, gather(1, 1)] if two_parts else [gather(1, 0)]
                for oc in range(c.OC):
                    o_proj_chunk(1, parts1, oc)

    nc.compile()
    return nc


# --------------------------------------------------------------------------
_CACHE = {}


def _get_program(cfg: Cfg, mode: str):
    key = (cfg.key(), mode)
    if key not in _CACHE:
        _CACHE[key] = build_program(cfg, mode)
    return _CACHE[key]


def prepare_inputs(cfg: Cfg, hidden_states, attention_mask, W_pack, W_o):
    """Host-side shard + layout prep (bf16 cast). Returns (mode, in_maps)."""
    c = cfg
    X = np.asarray(hidden_states, dtype=np.float32).reshape(c.T, c.hidden)
    XT = np.ascontiguousarray(X.T).astype(BF)

    mask = np.asarray(attention_mask, dtype=np.float32).reshape(c.S, c.S)
    causal_ref = np.where(
        np.tril(np.ones((c.S, c.S), dtype=bool)), 0.0, -1e9
    ).astype(np.float32)
    if np.array_equal(mask, causal_ref):
        mode = "causal"
    elif not mask.any():
        mode = "dense"
    else:
        mode = "masked"

    W_pack = np.asarray(W_pack, dtype=np.float32)
    W_o = np.asarray(W_o, dtype=np.float32)
    H = c.hidden
    # woT rows (features) reordered to the part-concatenated gather order:
    # for each head part, src-core-major then local head
    order = [s * c.HL + j
             for (h0, h1) in c.part_heads()
             for s in range(c.n_cores)
             for j in range(h0, h1)]
    woT = np.ascontiguousarray(
        W_o.T.reshape(c.n_heads, c.dh, c.hidden)[order]
        .reshape(c.hidden, c.hidden)).astype(BF)   # [feat, out] full
    in_maps = []
    for g in range(c.n_cores):
        r0, r1 = g * c.FO, (g + 1) * c.FO
        wq = W_pack[r0:r1]
        wk = W_pack[H + r0:H + r1]
        wv = W_pack[2 * H + r0:2 * H + r1]
        wqkvT = np.ascontiguousarray(
            np.concatenate([wq, wk, wv], axis=0).T).astype(BF)  # [H, F]
        m = {"xt": XT, "wqkvt": wqkvT, "wot": woT}
        if mode == "masked":
            m["maskt"] = np.ascontiguousarray(mask.T * math.sqrt(c.dh))
        in_maps.append(m)
    return mode, in_maps


def assemble_output(cfg: Cfg, results):
    c = cfg
    full = np.empty((c.B, c.S, c.hidden), dtype=np.float32)
    for g in range(c.n_cores):
        o = results[g]["out"].reshape(c.B, c.TSH, c.hidden)
        for b in range(c.B):
            full[b, g * c.TSH:(g + 1) * c.TSH] = o[b]
    return full


def kernel(hidden_states, attention_mask, W_pack, W_o):
    cfg = Cfg()
    mode, in_maps = prepare_inputs(cfg, hidden_states, attention_mask,
                                   W_pack, W_o)
    nc = _get_program(cfg, mode)
    res = bass_utils.run_bass_kernel_spmd(nc, in_maps,
                                          list(range(cfg.n_cores)))
    return assemble_output(cfg, res.results)


# revision 14
# speedup vs baseline: 1.0235x; 1.0097x over previous
"""Trainium2 Bass kernel for BaichuanAttention (hidden=5120, 40 heads, b=2, s=2048).

Tensor-parallel over heads across 8 NeuronCores, bf16 compute:
  A) QKV projection with SBUF-resident bf16 weights, X streamed.
  B) Flash-style causal attention in S^T form (scores computed as K^T.Q so
     exp() writes P^T directly -- no P transposes), V transposed on-chip.
     Score matmuls of chunk qc are interleaved with the PV matmuls of chunk
     qc-1 in PE program order so the exp() latency never gates the PE.
  C) Softmax-tail outputs are DMA'd straight into the AllToAll input DRAM
     buffer at flush time (features -> token shards); collectives trigger
     within ~2us of their heads finishing.  Gathers run on the sync queue,
     emitted late enough to never block it.  Local full-width o_proj per
     core on its token shard, interleaved with batch-1 attention.
Host reassembles the token-sharded outputs.
"""

import math
import sys

for _p in ("/opt/trn_rl_repo",):
    if _p not in sys.path:
        sys.path.insert(0, _p)

import numpy as np
import ml_dtypes

import concourse.bass as bass
import concourse.mybir as mybir
import concourse.tile as tile
from concourse import bacc, bass_utils

F32 = mybir.dt.float32
F32R = mybir.dt.float32r
BF16 = mybir.dt.bfloat16
BF = ml_dtypes.bfloat16


class Cfg:
    def __init__(self, hidden=5120, n_heads=40, dh=128, B=2, S=2048, n_cores=8):
        self.hidden = hidden
        self.n_heads = n_heads
        self.dh = dh
        self.B = B
        self.S = S
        self.n_cores = n_cores
        assert dh == 128
        self.HL = n_heads // n_cores          # heads per core (5)
        self.F = 3 * self.HL * dh             # per-core packed qkv rows (1920)
        self.FO = self.HL * dh                # per-core attn feature width (640)
        self.T = B * S                        # total tokens (4096)
        self.KC = hidden // 128               # contraction chunks (40)
        self.TC = self.T // 512               # token chunks for qkv (8)
        self.SKT = S // 128                   # k tiles per batch seq (16)
        self.QC = S // 512                    # q chunks per batch (4)
        self.NFT = self.F // 128              # qkv feature tiles (15)
        self.TSH = S // n_cores               # token shard per core per batch (256)
        self.OC = hidden // 512               # o_proj out chunks (10)

    def part_heads(self):
        if self.HL > 3:
            return [(0, 3), (3, self.HL)]
        return [(0, self.HL)]

    def key(self):
        return (self.hidden, self.n_heads, self.dh, self.B, self.S, self.n_cores)


def build_program(cfg: Cfg, mode: str):
    """mode: 'causal' (mult-mask diag blocks + block skip), 'dense' (no mask),
    'masked' (general additive mask, host passes maskT pre-scaled)."""
    assert mode in ("causal", "dense", "masked")
    c = cfg
    nc = bacc.Bacc("TRN2", target_bir_lowering=False, debug=False,
                   num_devices=c.n_cores)

    xt = nc.dram_tensor("xt", [c.hidden, c.T], BF16, kind="ExternalInput").ap()
    wqkvt = nc.dram_tensor("wqkvt", [c.hidden, c.F], BF16,
                           kind="ExternalInput").ap()
    wot = nc.dram_tensor("wot", [c.hidden, c.hidden], BF16,
                         kind="ExternalInput").ap()
    mask_ext = None
    if mode == "masked":
        mask_ext = nc.dram_tensor("maskt", [c.S, c.S], F32,
                                  kind="ExternalInput").ap()
    # per-core output: for each batch, this core's token shard (all hidden)
    out_ext = nc.dram_tensor("out", [c.B, c.TSH, c.hidden], F32,
                             kind="ExternalOutput").ap()

    inv_sqrt_dh = 1.0 / math.sqrt(c.dh)

    xt_r = xt.rearrange("(kc p) t -> p kc t", p=128)
    wq_r = wqkvt.rearrange("(kc p) f -> p kc f", p=128)
    wo_r = wot.rearrange("(kc p) j -> p kc j", p=128)

    part_heads = c.part_heads()
    two_parts = len(part_heads) > 1

    def part_of(h):
        for p, (h0, h1) in enumerate(part_heads):
            if h0 <= h < h1:
                return p, h0, h1
        raise AssertionError

    with tile.TileContext(nc) as tc:
        with tc.tile_pool(name="dram", bufs=1, space="DRAM") as dram:
            qkv = dram.tile([2 * c.HL, 128, c.T], BF16)
            vdram = dram.tile([c.B, c.SKT, 128, c.FO], BF16, tag="vdram",
                              name="vdram")
            a2a_in = {}
            a2a_out = {}
            for b in range(c.B):
                for p, (h0, h1) in enumerate(part_heads):
                    nh = h1 - h0
                    a2a_in[(b, p)] = dram.tile(
                        [c.n_cores, nh * 128, c.TSH], BF16,
                        tag=f"a2ai{b}{p}", name=f"a2ai{b}{p}")
                    a2a_out[(b, p)] = dram.tile(
                        [c.n_cores, nh * 128, c.TSH], BF16,
                        tag=f"a2ao{b}{p}", name=f"a2ao{b}{p}")

            # ---------------- Phase A: QKV projection -------------------
            # qkv[ft, d, t] = sum_h W[h, ft*128+d] * X[h, t]  (q,k feature-
            # major); V is produced token-major into vdram inside split 0,
            # reusing the resident xq tiles (no extra X pass).
            splits = [c.HL, c.HL]
            with tc.tile_pool(name="qkv_w", bufs=1) as wpool, \
                 tc.tile_pool(name="av_w", bufs=1) as wvpool, \
                 tc.tile_pool(name="qkv_x", bufs=2) as xpool, \
                 tc.tile_pool(name="qkv_o", bufs=8) as opool, \
                 tc.tile_pool(name="av_o", bufs=2) as ovpool, \
                 tc.tile_pool(name="qkv_ps", bufs=5, space="PSUM") as pspool, \
                 tc.tile_pool(name="av_pa", bufs=2, space="PSUM") as avpa, \
                 tc.tile_pool(name="av_pb", bufs=1, space="PSUM") as avpb:
                assert c.KC % 4 == 0
                KQ = c.KC // 4
                wvs = None
                ft0 = 0
                for si, nft in enumerate(splits):
                    wts = None
                    for tci in range(c.TC):
                        xq = [xpool.tile([128, KQ, 512], BF16, tag=f"x{j}",
                                         name=f"x{j}") for j in range(4)]
                        for j in range(4):
                            if si == 0 and tci == 0:
                                # sliced: the first matmul starts after one
                                # kc-slice lands instead of the whole tile
                                for kq in range(KQ):
                                    nc.sync.dma_start(
                                        xq[j][:, kq, :],
                                        xt_r[:, j * KQ + kq, 0:512])
                            else:
                                nc.sync.dma_start(
                                    xq[j][:],
                                    xt_r[:, j * KQ:(j + 1) * KQ,
                                         tci * 512:(tci + 1) * 512])
                        if tci == 0:
                            # per-kc weight tiles: lets the next group's
                            # weight loads overlap this group's tail
                            wts = []
                            for kc in range(c.KC):
                                w_t = wpool.tile([128, max(splits) * 128],
                                                 BF16, tag=f"w{kc}",
                                                 name=f"w{kc}")
                                nc.sync.dma_start(
                                    w_t[:, :nft * 128],
                                    wq_r[:, kc,
                                         ft0 * 128:(ft0 + nft) * 128])
                                wts.append(w_t)
                        if si == 0 and tci == 0:
                            # V weights, queued behind the first x/w tiles
                            wvs = []
                            for kc in range(c.KC):
                                wv_t = wvpool.tile([128, c.FO], BF16,
                                                   tag=f"wv{kc}",
                                                   name=f"wv{kc}")
                                nc.sync.dma_start(
                                    wv_t[:], wq_r[:, kc, 2 * c.FO:3 * c.FO])
                                wvs.append(wv_t)
                        pss = [pspool.tile([128, 512], F32, tag="ps",
                                           name=f"ps{i}")
                               for i in range(nft)]
                        for kc in range(c.KC):
                            for i in range(nft):
                                nc.tensor.matmul(
                                    pss[i][:],
                                    wts[kc][:, i * 128:(i + 1) * 128],
                                    xq[kc // KQ][:, kc % KQ, :],
                                    start=(kc == 0), stop=(kc == c.KC - 1))
                        for i in range(nft):
                            o_sb = opool.tile([128, 512], BF16, tag="o")
                            nc.vector.tensor_copy(o_sb[:], pss[i][:])
                            nc.sync.dma_start(
                                qkv[ft0 + i, :, tci * 512:(tci + 1) * 512],
                                o_sb[:])
                        if si == 0:
                            # V token-major for this tc's 4 token tiles:
                            # vdram[b, st, tok_p, f] = sum_h X[h,tok] Wv[h,f]
                            for j2 in range(4):
                                tt = tci * 4 + j2
                                vb, st = tt // c.SKT, tt % c.SKT
                                ps_a = avpa.tile([128, 512], F32, tag="pa",
                                                 name="ps_a")
                                ps_b = avpb.tile([128, c.FO - 512], F32,
                                                 tag="pb", name="ps_b")
                                for kc in range(c.KC):
                                    xsl = xq[kc // KQ][:, kc % KQ,
                                              j2 * 128:(j2 + 1) * 128]
                                    nc.tensor.matmul(
                                        ps_a[:], xsl, wvs[kc][:, :512],
                                        start=(kc == 0),
                                        stop=(kc == c.KC - 1))
                                    nc.tensor.matmul(
                                        ps_b[:], xsl, wvs[kc][:, 512:],
                                        start=(kc == 0),
                                        stop=(kc == c.KC - 1))
                                ov = ovpool.tile([128, c.FO], BF16,
                                                 tag="ov", name="ov")
                                nc.vector.tensor_copy(ov[:, :512], ps_a[:])
                                nc.scalar.activation(
                                    ov[:, 512:], ps_b[:],
                                    mybir.ActivationFunctionType.Copy)
                                nc.sync.dma_start(vdram[vb, st], ov[:])
                    ft0 += nft

            # ---------------- Phase B + C (interleaved per batch) -------
            npt = (2 * c.SKT - 3) if mode == "causal" else (2 * c.SKT + 1)
            with tc.tile_pool(name="att_const", bufs=1) as cpool, \
                 tc.tile_pool(name="att_in", bufs=2) as inpool, \
                 tc.tile_pool(name="att_v", bufs=2) as vpool, \
                 tc.tile_pool(name="att_pt", bufs=npt) as ptpool, \
                 tc.tile_pool(name="att_acc", bufs=2) as accpool, \
                 tc.tile_pool(name="att_sm", bufs=2) as smpool, \
                 tc.tile_pool(name="att_fl", bufs=3) as flpool, \
                 tc.tile_pool(name="att_ms", bufs=(4 if mode == "masked" else 1)) as mspool, \
                 tc.tile_pool(name="op_attn", bufs=2) as apool, \
                 tc.tile_pool(name="op_w", bufs=6) as wopool, \
                 tc.tile_pool(name="op_o", bufs=3) as oopool, \
                 tc.tile_pool(name="ps_s", bufs=2, space="PSUM") as ps_s, \
                 tc.tile_pool(name="ps_at", bufs=2, space="PSUM") as ps_at, \
                 tc.tile_pool(name="ps_ms", bufs=2, space="PSUM") as ps_ms, \
                 tc.tile_pool(name="ps_op", bufs=2, space="PSUM") as ps_op:

                # constants
                ones_col_b = cpool.tile([128, 1], BF16)   # lhsT for pt colsum
                ones_col_r = cpool.tile([128, 1], F32R)   # lhsT for acc merge
                ones_row = cpool.tile([1, 128], F32R)     # lhsT for broadcast
                with tc.tile_pool(name="att_tmp", bufs=1) as tmppool:
                    o32 = tmppool.tile([128, 1], F32, tag="o32")
                    nc.vector.memset(o32[:], 1.0)
                    nc.vector.tensor_copy(ones_col_b[:], o32[:])
                    nc.vector.tensor_copy(ones_col_r[:], o32[:])
                    r32 = tmppool.tile([1, 128], F32, tag="r32")
                    nc.vector.memset(r32[:], 1.0)
                    nc.vector.tensor_copy(ones_row[:], r32[:])
                ctri = None
                if mode == "causal":
                    # multiplicative triangle mask [128k, 128q]:
                    # m[p, y] = 1 where y >= p else 0
                    with tc.tile_pool(name="att_cm", bufs=1) as cmtmp:
                        m32 = cmtmp.tile([128, 128], F32, tag="m32")
                        nc.gpsimd.memset(m32[:], 1.0)
                        nc.gpsimd.affine_select(
                            out=m32[:], in_=m32[:],
                            compare_op=mybir.AluOpType.is_ge, fill=0.0,
                            base=0, pattern=[[1, 128]],
                            channel_multiplier=-1)
                        ctri = cpool.tile([128, 128], BF16, tag="ctri")
                        nc.vector.tensor_copy(ctri[:], m32[:])

                pending = []

                def emit_pv(prev, j):
                    # one PV matmul of the deferred (previous) chunk
                    off = j - 4 * prev["qc"]
                    q0 = off * 128 if (mode == "causal" and off > 0) else 0
                    nkt_p = len(prev["pts"])
                    nc.tensor.matmul(
                        prev["at_ps"][:, q0:], prev["v_tok"][:, j, :],
                        prev["pts"][j][:, q0:],
                        start=(j == 0), stop=(j == nkt_p - 1))

                def early_finish(prev):
                    # reciprocal path of the previous chunk's denominator:
                    # [1,512] PSUM -> SBUF on scalar, broadcast via PE,
                    # full-parallel reciprocal on vector
                    den_sb = smpool.tile([1, 512], F32R, tag="densb",
                                         name="den_sb")
                    nc.scalar.activation(den_sb[:], prev["den_ps"][:],
                                         mybir.ActivationFunctionType.Copy)
                    bc_ps = ps_ms.tile([128, 512], F32, tag="ms",
                                       name="bc_ps")
                    nc.tensor.matmul(bc_ps[:], ones_row[:], den_sb[:],
                                     start=True, stop=True)
                    rbc = smpool.tile([128, 512], F32, tag="rbc", name="rbc")
                    nc.vector.reciprocal_approx_fast(rbc[:], bc_ps[:])
                    prev["rbc"] = rbc

                def late_finish(prev):
                    # normalized output chunk + staging DMA into the
                    # AllToAll input buffer (token-shard layout)
                    ofl = flpool.tile([128, 512], BF16, tag="ofl",
                                      name="ofl")
                    nc.vector.tensor_tensor(ofl[:], prev["at_ps"][:],
                                            prev["rbc"][:],
                                            mybir.AluOpType.mult)
                    p, h0, _ = part_of(prev["h"])
                    qc_p = prev["qc"]
                    dst = a2a_in[(prev["b"], p)].rearrange(
                        "s (f q) t -> q s f t", q=128)[
                        :, 2 * qc_p:2 * qc_p + 2, prev["h"] - h0, :]
                    nc.sync.dma_start(
                        dst, ofl[:].rearrange("q (s t) -> q s t", s=2))

                def flush_serial():
                    if not pending:
                        return
                    prev = pending.pop()
                    at_ps = ps_at.tile([128, 512], F32, tag="at",
                                       name="at_ps")
                    prev["at_ps"] = at_ps
                    early_finish(prev)
                    for j in range(len(prev["pts"])):
                        emit_pv(prev, j)
                    late_finish(prev)

                def attend_head(b, h):
                    t0 = b * c.S
                    q_sb = inpool.tile([128, c.S], BF16, tag="q",
                                       name="q_sb")
                    k_sb = inpool.tile([128, c.S], BF16, tag="k",
                                       name="k_sb")
                    nc.sync.dma_start(q_sb[:], qkv[h, :, t0:t0 + c.S])
                    nc.sync.dma_start(k_sb[:], qkv[c.HL + h, :, t0:t0 + c.S])
                    v_tok = vpool.tile([128, c.SKT, 128], BF16, tag="vt",
                                       name="v_tok")
                    nc.sync.dma_start(
                        v_tok[:],
                        vdram[b].rearrange("s p d -> p s d")[
                            :, :, h * 128:(h + 1) * 128])

                    for qc in range(c.QC):
                        nkt = 4 * (qc + 1) if mode == "causal" else c.SKT
                        prev = pending.pop() if pending else None
                        if prev is not None:
                            at_ps = ps_at.tile([128, 512], F32, tag="at",
                                               name="at_ps")
                            prev["at_ps"] = at_ps
                        den_ps = ps_ms.tile([1, 512], F32, tag="ms",
                                            name="den_ps")
                        acc_v = accpool.tile([128, 512], F32R, tag="accv",
                                             name="acc_v")
                        pnk = len(prev["pts"]) if prev is not None else 0
                        emitted = 0
                        first_pe = True
                        pts = []
                        for kt in range(nkt):
                            off = kt - 4 * qc  # >=0: diagonal tile (causal)
                            pt = ptpool.tile([128, 512], BF16, tag="pt",
                                             name="pt")
                            s_ps = ps_s.tile([128, 512], F32, tag="s",
                                             name="s_ps")
                            if mode == "causal" and off > 0:
                                # valid q range is [off*128, 512); the
                                # region below is never read (trimmed)
                                q0 = off * 128
                                w = 512 - q0
                                nc.tensor.matmul(
                                    s_ps[:, :w],
                                    k_sb[:, kt * 128:(kt + 1) * 128],
                                    q_sb[:, qc * 512 + q0:(qc + 1) * 512],
                                    start=True, stop=True)
                                nc.scalar.activation(
                                    pt[:, q0:], s_ps[:, :w],
                                    mybir.ActivationFunctionType.Exp,
                                    scale=inv_sqrt_dh)
                                nc.vector.tensor_tensor(
                                    pt[:, q0:q0 + 128],
                                    pt[:, q0:q0 + 128],
                                    ctri[:], mybir.AluOpType.mult)
                            else:
                                q0 = 0
                                nc.tensor.matmul(
                                    s_ps[:],
                                    k_sb[:, kt * 128:(kt + 1) * 128],
                                    q_sb[:, qc * 512:(qc + 1) * 512],
                                    start=True, stop=True)
                                if mode == "masked":
                                    m_sb = mspool.tile([128, 512], F32,
                                                       tag="m", name="m_sb")
                                    nc.sync.dma_start(
                                        m_sb[:],
                                        mask_ext[kt * 128:(kt + 1) * 128,
                                                 qc * 512:(qc + 1) * 512])
                                    nc.vector.tensor_tensor(
                                        s_ps[:], s_ps[:], m_sb[:],
                                        mybir.AluOpType.add)
                                nc.scalar.activation(
                                    pt[:], s_ps[:],
                                    mybir.ActivationFunctionType.Exp,
                                    scale=inv_sqrt_dh)
                                if mode == "causal" and off == 0:
                                    nc.vector.tensor_tensor(
                                        pt[:, :128], pt[:, :128],
                                        ctri[:], mybir.AluOpType.mult)
                            # denominator: even tiles on a vector chain,
                            # odd tiles as PE colsum matmuls
                            if kt % 2 == 0:
                                if kt == 0:
                                    nc.vector.tensor_copy(acc_v[:], pt[:])
                                else:
                                    nc.vector.tensor_tensor(
                                        acc_v[:, q0:], acc_v[:, q0:],
                                        pt[:, q0:], mybir.AluOpType.add)
                            else:
                                nc.tensor.matmul(
                                    den_ps[:, q0:], ones_col_b[:],
                                    pt[:, q0:],
                                    start=first_pe, stop=False)
                                first_pe = False
                            pts.append(pt)
                            # interleave the deferred PV matmuls of the
                            # previous chunk between this chunk's score
                            # matmuls: PE stays busy while exp() drains
                            if prev is not None:
                                tgt = ((kt + 1) * pnk) // nkt
                                while emitted < tgt:
                                    emit_pv(prev, emitted)
                                    emitted += 1
                                if kt == 0:
                                    early_finish(prev)
                        # fold the vector chain into the denominator
                        nc.tensor.matmul(den_ps[:], ones_col_r[:], acc_v[:],
                                         start=first_pe, stop=True)
                        if prev is not None:
                            late_finish(prev)
                        pending.append(dict(
                            b=b, h=h, qc=qc, den_ps=den_ps, pts=pts,
                            v_tok=v_tok))

                def launch_a2a(b, p):
                    flush_serial()
                    nc.gpsimd.collective_compute(
                        "AllToAll",
                        mybir.AluOpType.bypass,
                        replica_groups=[list(range(c.n_cores))],
                        ins=[a2a_in[(b, p)][:].opt()],
                        outs=[a2a_out[(b, p)][:].opt()],
                    )

                def gather(b, p):
                    h0, h1 = part_heads[p]
                    nh = h1 - h0
                    attn_sb = apool.tile([128, c.n_cores * nh, c.TSH], BF16,
                                         tag=f"ag{p}", name=f"ag{b}{p}")
                    nc.sync.dma_start(
                        attn_sb[:],
                        a2a_out[(b, p)].rearrange("s (f q) t -> q (s f) t",
                                                  q=128))
                    return attn_sb

                # per-oc contraction layout: (part, fc offset in part, count)
                wo_layout = []
                fc0 = 0
                for p, (h0, h1) in enumerate(part_heads):
                    nfc = c.n_cores * (h1 - h0)
                    ka = nfc // 2
                    wo_layout.append((p, 0, ka, fc0))
                    wo_layout.append((p, ka, nfc - ka, fc0 + ka))
                    fc0 += nfc

                max_cnt = max(le[2] for le in wo_layout)

                def o_proj_chunk(b, parts, oc):
                    wo_sbs = []
                    for (p, k0, cnt, gfc) in wo_layout:
                        wo_t = wopool.tile([128, max_cnt, 512], BF16,
                                           tag="wo", name="wo_t")
                        nc.sync.dma_start(
                            wo_t[:, :cnt, :],
                            wo_r[:, gfc:gfc + cnt,
                                 oc * 512:(oc + 1) * 512])
                        wo_sbs.append(wo_t)
                    last = len(wo_layout) - 1
                    for tt in range(c.TSH // 128):
                        ps = ps_op.tile([128, 512], F32, tag="ops",
                                        name="op_ps")
                        for wi, (p, k0, cnt, gfc) in enumerate(wo_layout):
                            for k in range(cnt):
                                nc.tensor.matmul(
                                    ps[:],
                                    parts[p][:, k0 + k,
                                             tt * 128:(tt + 1) * 128],
                                    wo_sbs[wi][:, k, :],
                                    start=(wi == 0 and k == 0),
                                    stop=(wi == last and k == cnt - 1))
                        po_sb = oopool.tile([128, 512], F32, tag="po",
                                            name="po_sb")
                        nc.vector.tensor_copy(po_sb[:], ps[:])
                        nc.gpsimd.dma_start(
                            out_ext[b, tt * 128:(tt + 1) * 128,
                                    oc * 512:(oc + 1) * 512],
                            po_sb[:])

                # ---- schedule ----
                h_p0 = part_heads[0][1] - 1
                # batch 0 attention; part collectives trigger right after
                # their last head's tail is flushed
                for h in range(c.HL):
                    attend_head(0, h)
                    if two_parts and h == h_p0:
                        launch_a2a(0, 0)
                launch_a2a(0, 1 if two_parts else 0)
                # batch 1 attention interleaved with batch-0 o_proj
                attend_head(1, 0)
                g00 = gather(0, 0)
                attend_head(1, 1)
                parts0 = [g00, gather(0, 1)] if two_parts else [g00]
                attend_head(1, 2)
                if two_parts:
                    launch_a2a(1, 0)
                attend_head(1, 3)
                o_proj_chunk(0, parts0, 0)
                o_proj_chunk(0, parts0, 1)
                attend_head(1, 4)
                if two_parts:
                    launch_a2a(1, 1)
                for oc in range(2, c.OC):
                    o_proj_chunk(0, parts0, oc)
                if two_parts:
                    g10 = gather(1, 0)
                parts1 = [g10, gather(1, 1)] if two_parts else [gather(1, 0)]
                for oc in range(c.OC):
                    o_proj_chunk(1, parts1, oc)

    nc.compile()
    return nc


# --------------------------------------------------------------------------
_CACHE = {}


def _get_program(cfg: Cfg, mode: str):
    key = (cfg.key(), mode)
    if key not in _CACHE:
        _CACHE[key] = build_program(cfg, mode)
    return _CACHE[key]


def prepare_inputs(cfg: Cfg, hidden_states, attention_mask, W_pack, W_o):
    """Host-side shard + layout prep (bf16 cast). Returns (mode, in_maps)."""
    c = cfg
    X = np.asarray(hidden_states, dtype=np.float32).reshape(c.T, c.hidden)
    XT = np.ascontiguousarray(X.T).astype(BF)

    mask = np.asarray(attention_mask, dtype=np.float32).reshape(c.S, c.S)
    causal_ref = np.where(
        np.tril(np.ones((c.S, c.S), dtype=bool)), 0.0, -1e9
    ).astype(np.float32)
    if np.array_equal(mask, causal_ref):
        mode = "causal"
    elif not mask.any():
        mode = "dense"
    else:
        mode = "masked"

    W_pack = np.asarray(W_pack, dtype=np.float32)
    W_o = np.asarray(W_o, dtype=np.float32)
    H = c.hidden
    # woT rows (features) reordered to the part-concatenated gather order:
    # for each head part, src-core-major then local head
    order = [s * c.HL + j
             for (h0, h1) in c.part_heads()
             for s in range(c.n_cores)
             for j in range(h0, h1)]
    woT = np.ascontiguousarray(
        W_o.T.reshape(c.n_heads, c.dh, c.hidden)[order]
        .reshape(c.hidden, c.hidden)).astype(BF)   # [feat, out] full
    in_maps = []
    for g in range(c.n_cores):
        r0, r1 = g * c.FO, (g + 1) * c.FO
        wq = W_pack[r0:r1]
        wk = W_pack[H + r0:H + r1]
        wv = W_pack[2 * H + r0:2 * H + r1]
        wqkvT = np.ascontiguousarray(
            np.concatenate([wq, wk, wv], axis=0).T).astype(BF)  # [H, F]
        m = {"xt": XT, "wqkvt": wqkvT, "wot": woT}
        if mode == "masked":
            m["maskt"] = np.ascontiguousarray(mask.T * math.sqrt(c.dh))
        in_maps.append(m)
    return mode, in_maps


def assemble_output(cfg: Cfg, results):
    c = cfg
    full = np.empty((c.B, c.S, c.hidden), dtype=np.float32)
    for g in range(c.n_cores):
        o = results[g]["out"].reshape(c.B, c.TSH, c.hidden)
        for b in range(c.B):
            full[b, g * c.TSH:(g + 1) * c.TSH] = o[b]
    return full


def kernel(hidden_states, attention_mask, W_pack, W_o):
    cfg = Cfg()
    mode, in_maps = prepare_inputs(cfg, hidden_states, attention_mask,
                                   W_pack, W_o)
    nc = _get_program(cfg, mode)
    res = bass_utils.run_bass_kernel_spmd(nc, in_maps,
                                          list(range(cfg.n_cores)))
    return assemble_output(cfg, res.results)
